# revision 13
# baseline (speedup 1.0000x reference)
"""SigLIP loss kernel for 8 Trainium2 NeuronCores.

Strategy:
  - Row-shard video_embed across the 8 cores (1024 rows each); each core keeps
    the full text_embed in its own HBM (HBM reads beat ring-exchange link BW).
  - Each core computes its 1024x8192 block of logits with bf16 matmuls
    (fp32 PSUM accumulate), applies Softplus on the ScalarEngine with fused
    per-row accumulation, and tracks the per-row max on the VectorEngine.
  - Host: computes inverse row norms (tiny O(N*D) prep), assembles
    loss = (sum softplus(l_ij) - trace) / N, and resolves the argmax-accuracy
    with an exact float64 recheck of the few rows whose diag-vs-rowmax margin
    is within a small band (bf16 can't flip confident rows).
"""

import os
from contextlib import ExitStack

import numpy as np

N, D = 8192, 768
P = 128           # SBUF partitions
KC = D // P       # 6 contraction chunks
NCORES = 8
NV = N // NCORES  # 1024 v rows per core
NVB = NV // P     # 8 v blocks per core
TBW = 512         # t-block width (matmul free dim)
NTB = N // TBW    # 16 t blocks
SUB = TBW // P    # 4 sub-blocks of 128 t rows per t block
MARGIN_BAND = 0.05

_COMPILED = None


def _build_nc():
    import concourse.bass as bass  # noqa: F401
    import concourse.mybir as mybir
    import concourse.tile as tile
    from concourse import bacc

    f32 = mybir.dt.float32
    bf16 = mybir.dt.bfloat16
    EXP = mybir.ActivationFunctionType.Exp
    LN = mybir.ActivationFunctionType.Ln
    AX = mybir.AxisListType.X
    AXY = mybir.AxisListType.XY

    nc = bacc.Bacc(
        "TRN2",
        target_bir_lowering=False,
        debug=False,
        enable_asserts=False,
        num_devices=NCORES,
    )

    # The act-table placement pass picks the first table set containing each
    # function; Exp and Ln individually resolve to different sets, which
    # forces a ~1.3us ACT_TABLE_LOAD per activation. Hide Exp/Ln from every
    # set except the combined one (set order/indices preserved) so both
    # resolve to natural_log_exp_and_others and a single load suffices.
    orig_tables = dict(bacc.get_activation_tables(nc.m.arch))
    patched = {
        name: (fns if name == "natural_log_exp_and_others" else fns - {EXP, LN})
        for name, fns in orig_tables.items()
    }
    bacc.get_activation_tables = lambda arch: patched

    v_d = nc.dram_tensor("v", [NV, D], f32, kind="ExternalInput")
    t_d = nc.dram_tensor("t", [N, D], f32, kind="ExternalInput")
    invv_d = nc.dram_tensor("inv_v", [P, NVB], f32, kind="ExternalInput")
    invt_d = nc.dram_tensor("inv_t", [P, N // P], f32, kind="ExternalInput")
    rs_d = nc.dram_tensor("row_sum", [P, NVB], f32, kind="ExternalOutput")
    rm_d = nc.dram_tensor("row_max", [P, NVB], f32, kind="ExternalOutput")

    with tile.TileContext(nc) as tc, ExitStack() as ctx:
        singles = ctx.enter_context(tc.tile_pool(name="singles", bufs=1))
        vstage = ctx.enter_context(tc.tile_pool(name="vstage", bufs=1))
        tstage = ctx.enter_context(tc.tile_pool(name="tstage", bufs=2))
        tbfp = ctx.enter_context(tc.tile_pool(name="tbfp", bufs=3))
        ttp = ctx.enter_context(tc.tile_pool(name="ttp", bufs=3))
        spp = ctx.enter_context(tc.tile_pool(name="spp", bufs=2))
        psum_mm = ctx.enter_context(tc.tile_pool(name="psum_mm", bufs=4, space="PSUM"))

        invv = singles.tile([P, NVB], f32)
        nc.gpsimd.dma_start(out=invv, in_=invv_d.ap())
        invt = singles.tile([P, N // P], f32)
        nc.gpsimd.dma_start(out=invt, in_=invt_d.ap())

        # ---- v prep: load, scale+cast to bf16, transpose to [d, row] layout
        # (transposes ride the DMA xbar, keeping the PE free for matmuls)
        vst = vstage.tile([P, NVB, D], f32)
        nc.gpsimd.dma_start(out=vst, in_=v_d.ap().rearrange("(a p) d -> p a d", p=P))
        vbf = vstage.tile([P, NVB, D], bf16)
        for vb in range(NVB):
            nc.vector.tensor_scalar_mul(
                vbf[:, vb, :], vst[:, vb, :], invv[:, vb : vb + 1]
            )
        vT = singles.tile([P, KC, NV], bf16)
        for vb in range(NVB):
            for k in range(KC):
                nc.sync.dma_start_transpose(
                    vT[:, k, vb * P : (vb + 1) * P],
                    vbf[:, vb, k * P : (k + 1) * P],
                )

        NT2 = N // (2 * TBW)  # 8 column-blocks of 1024 t rows
        SUB2 = 2 * TBW // P  # 8 row sub-blocks per column-block
        rs_cols = singles.tile([P, NVB, NT2], f32)
        mx_cols = singles.tile([P, NVB, NT2], f32)

        # ---- main loop over t blocks of 1024 rows
        for tb in range(NT2):
            tst = tstage.tile([P, SUB2, D], f32)
            nc.gpsimd.dma_start(
                out=tst,
                in_=t_d.ap()[tb * 1024 : (tb + 1) * 1024, :].rearrange(
                    "(a p) d -> p a d", p=P
                ),
            )
            ttb = ttp.tile([P, KC, 2 * TBW], bf16)
            for sb in range(SUB2):
                idx = tb * SUB2 + sb
                tb16 = tbfp.tile([P, D], bf16)
                nc.vector.tensor_scalar_mul(
                    tb16, tst[:, sb, :], invt[:, idx : idx + 1]
                )
                for k in range(KC):
                    nc.sync.dma_start_transpose(
                        ttb[:, k, sb * P : (sb + 1) * P],
                        tb16[:, k * P : (k + 1) * P],
                    )

            for vb in range(NVB):
                ps = psum_mm.tile([P, 2, TBW], f32)
                for h in range(2):
                    for k in range(KC):
                        nc.tensor.matmul(
                            ps[:, h, :],
                            vT[:, k, vb * P : (vb + 1) * P],
                            ttb[:, k, h * TBW : (h + 1) * TBW],
                            start=(k == 0),
                            stop=(k == KC - 1),
                        )
                # softplus(x) = ln(exp(x) + 1); this toolchain has no Softplus
                # table, but Exp and Ln live in one table set. The bf16 exp
                # scratch also carries the row-max (exp is monotone; host
                # recovers the logit-space max with a log).
                ex = spp.tile([P, 2, TBW], bf16)
                nc.scalar.activation(ex, ps, EXP)
                sp = spp.tile([P, 2, TBW], bf16, tag="sp_dead")
                nc.scalar.activation(
                    sp, ex, LN, bias=1.0, accum_out=rs_cols[:, vb, tb : tb + 1]
                )
                nc.vector.tensor_reduce(
                    mx_cols[:, vb, tb : tb + 1],
                    ex,
                    axis=AXY,
                    op=mybir.AluOpType.max,
                )

        rs_out = singles.tile([P, NVB], f32)
        mx_out = singles.tile([P, NVB], f32)
        for vb in range(NVB):
            nc.vector.reduce_sum(rs_out[:, vb : vb + 1], rs_cols[:, vb, :], axis=AX)
            nc.vector.reduce_max(mx_out[:, vb : vb + 1], mx_cols[:, vb, :], axis=AX)
        nc.sync.dma_start(out=rs_d.ap(), in_=rs_out)
        nc.sync.dma_start(out=rm_d.ap(), in_=mx_out)

    nc.compile()
    return nc


def _get_compiled():
    global _COMPILED
    if _COMPILED is None:
        _COMPILED = _build_nc()
    return _COMPILED


def _run_device(v64, t64, inv_v, inv_t, trace=False):
    from concourse.bass_utils import run_bass_kernel_spmd

    nc = _get_compiled()
    v32 = v64.astype(np.float32)
    t32 = t64.astype(np.float32)
    in_maps = []
    for c in range(NCORES):
        iv = inv_v[c * NV : (c + 1) * NV]  # [1024]
        in_maps.append(
            {
                "v": np.ascontiguousarray(v32[c * NV : (c + 1) * NV]),
                "t": t32,
                # [p, vb] layout: row vb*128+p
                "inv_v": np.ascontiguousarray(
                    iv.reshape(NVB, P).T.astype(np.float32)
                ),
                "inv_t": np.ascontiguousarray(
                    inv_t.reshape(N // P, P).T.astype(np.float32)
                ),
            }
        )
    res = run_bass_kernel_spmd(
        nc, in_maps, core_ids=list(range(NCORES)), trace=trace
    )
    return res


def kernel(video_embed, text_embed, log_logit_scale, _trace=False, _res_out=None):
    video_embed = np.asarray(video_embed)
    text_embed = np.asarray(text_embed)
    scale = float(np.exp(np.float64(np.asarray(log_logit_scale))))

    v64 = video_embed.astype(np.float64)
    t64 = text_embed.astype(np.float64)
    vn = np.linalg.norm(v64, axis=1)
    tn = np.linalg.norm(t64, axis=1)
    inv_v = (scale / vn).astype(np.float64)  # scale folded into v rows
    inv_t = (1.0 / tn).astype(np.float64)

    res = _run_device(v64, t64, inv_v, inv_t, trace=_trace)
    if _res_out is not None:
        _res_out.append(res)

    row_sum = np.concatenate(
        [res.results[c]["row_sum"].T.reshape(-1) for c in range(NCORES)]
    ).astype(np.float64)
    row_max_exp = np.concatenate(
        [res.results[c]["row_max"].T.reshape(-1) for c in range(NCORES)]
    ).astype(np.float64)
    # device tracks max of exp(logit) (monotone); recover logit-space max
    row_max = np.log(np.maximum(row_max_exp, 1e-300))

    # host: exact diag & trace in f64
    v_hat = v64 / vn[:, None]
    t_hat = t64 / tn[:, None]
    diag = scale * np.einsum("ij,ij->i", v_hat, t_hat)
    S = row_sum.sum()
    loss = (S - diag.sum()) / N

    # accuracy: rows where the diagonal could possibly be the argmax get an
    # exact recheck; everything else is confidently not a match.
    cand = np.nonzero(diag >= row_max - MARGIN_BAND)[0]
    k = 0
    for i in cand:
        row = scale * (t_hat @ v_hat[i])
        row[i] = diag[i]
        if int(np.argmax(row)) == i:
            k += 1
    acc = 100.0 * k / N

    return np.float32(loss), np.float32(acc)


# revision 21
# speedup vs baseline: 2.0824x; 2.0824x over previous
"""SigLIP loss kernel for 8 Trainium2 NeuronCores.

Strategy:
  - Row-shard video_embed across the 8 cores (1024 rows each); each core keeps
    the full text_embed in its own HBM (HBM reads beat ring-exchange link BW).
  - Each core computes its 1024x8192 block of logits with bf16 matmuls
    (fp32 PSUM accumulate), applies Softplus on the ScalarEngine with fused
    per-row accumulation, and tracks the per-row max on the VectorEngine.
  - Host: computes inverse row norms (tiny O(N*D) prep), assembles
    loss = (sum softplus(l_ij) - trace) / N, and resolves the argmax-accuracy
    with an exact float64 recheck of the few rows whose diag-vs-rowmax margin
    is within a small band (bf16 can't flip confident rows).
"""

import os
from contextlib import ExitStack

import numpy as np

N, D = 8192, 768
P = 128           # SBUF partitions
KC = D // P       # 6 contraction chunks
NCORES = 8
NV = N // NCORES  # 1024 v rows per core
NVB = NV // P     # 8 v blocks per core
TBW = 512         # t-block width (matmul free dim)
NTB = N // TBW    # 16 t blocks
SUB = TBW // P    # 4 sub-blocks of 128 t rows per t block
MARGIN_BAND = 0.05

_COMPILED = None


def _build_nc():
    import concourse.bass as bass  # noqa: F401
    import concourse.mybir as mybir
    import concourse.tile as tile
    from concourse import bacc

    f32 = mybir.dt.float32
    bf16 = mybir.dt.bfloat16
    EXP = mybir.ActivationFunctionType.Exp
    LN = mybir.ActivationFunctionType.Ln
    AX = mybir.AxisListType.X
    AXY = mybir.AxisListType.XY

    nc = bacc.Bacc(
        "TRN2",
        target_bir_lowering=False,
        debug=False,
        enable_asserts=False,
        num_devices=NCORES,
    )

    # The act-table placement pass picks the first table set containing each
    # function; Exp and Ln individually resolve to different sets, which
    # forces a ~1.3us ACT_TABLE_LOAD per activation. Hide Exp/Ln from every
    # set except the combined one (set order/indices preserved) so both
    # resolve to natural_log_exp_and_others and a single load suffices.
    orig_tables = dict(bacc.get_activation_tables(nc.m.arch))
    patched = {
        name: (fns if name == "natural_log_exp_and_others" else fns - {EXP, LN})
        for name, fns in orig_tables.items()
    }
    bacc.get_activation_tables = lambda arch: patched

    v_d = nc.dram_tensor("v", [NV, D], f32, kind="ExternalInput")
    t_d = nc.dram_tensor("t", [N, D], f32, kind="ExternalInput")
    invv_d = nc.dram_tensor("inv_v", [P, NVB], f32, kind="ExternalInput")
    invt_d = nc.dram_tensor("inv_t", [P, N // P], f32, kind="ExternalInput")
    rs_d = nc.dram_tensor("row_sum", [P, NVB], f32, kind="ExternalOutput")
    rm_d = nc.dram_tensor("row_max", [P, NVB], f32, kind="ExternalOutput")

    with tile.TileContext(nc) as tc, ExitStack() as ctx:
        singles = ctx.enter_context(tc.tile_pool(name="singles", bufs=1))
        vstage = ctx.enter_context(tc.tile_pool(name="vstage", bufs=1))
        tstage = ctx.enter_context(tc.tile_pool(name="tstage", bufs=2))
        tbfp = ctx.enter_context(tc.tile_pool(name="tbfp", bufs=3))
        ttp = ctx.enter_context(tc.tile_pool(name="ttp", bufs=3))
        spp = ctx.enter_context(tc.tile_pool(name="spp", bufs=2))
        psum_mm = ctx.enter_context(tc.tile_pool(name="psum_mm", bufs=3, space="PSUM"))
        psum_tr = ctx.enter_context(tc.tile_pool(name="psum_tr", bufs=2, space="PSUM"))

        from concourse.masks import make_identity

        ident = singles.tile([P, P], bf16)
        make_identity(nc, ident)

        invv = singles.tile([P, NVB], f32)
        nc.gpsimd.dma_start(out=invv, in_=invv_d.ap())
        invt = singles.tile([P, N // P], f32)
        nc.gpsimd.dma_start(out=invt, in_=invt_d.ap())

        # ---- v prep: load, scale+cast to bf16, transpose to [d, row] layout
        # (transposes ride the DMA xbar, keeping the PE free for matmuls)
        vst = vstage.tile([P, NVB, D], f32)
        nc.gpsimd.dma_start(out=vst, in_=v_d.ap().rearrange("(a p) d -> p a d", p=P))
        vbf = vstage.tile([P, NVB, D], bf16)
        for vb in range(NVB):
            nc.vector.tensor_scalar_mul(
                vbf[:, vb, :], vst[:, vb, :], invv[:, vb : vb + 1]
            )
        vT = singles.tile([P, KC, NV], bf16)
        for vb in range(NVB):
            for k in range(KC):
                pt = psum_tr.tile([P, P], bf16)
                nc.tensor.transpose(pt, vbf[:, vb, k * P : (k + 1) * P], ident)
                nc.vector.tensor_copy(vT[:, k, vb * P : (vb + 1) * P], pt)

        NT2 = N // (2 * TBW)  # 8 column-blocks of 1024 t rows
        SUB2 = 2 * TBW // P  # 8 row sub-blocks per column-block
        rs_cols = singles.tile([P, NVB, NT2], f32)
        mx_cols = singles.tile([P, NVB, NT2], bf16)

        # ---- main loop over t blocks of 1024 rows
        for tb in range(NT2):
            tst = tstage.tile([P, SUB2, D], f32)
            nc.gpsimd.dma_start(
                out=tst,
                in_=t_d.ap()[tb * 1024 : (tb + 1) * 1024, :].rearrange(
                    "(a p) d -> p a d", p=P
                ),
            )
            ttb = ttp.tile([P, KC, 2 * TBW], bf16)
            for sb in range(SUB2):
                idx = tb * SUB2 + sb
                tb16 = tbfp.tile([P, D], bf16)
                nc.vector.tensor_scalar_mul(
                    tb16, tst[:, sb, :], invt[:, idx : idx + 1]
                )
                for k in range(KC):
                    pt = psum_tr.tile([P, P], bf16)
                    nc.tensor.transpose(pt, tb16[:, k * P : (k + 1) * P], ident)
                    nc.vector.tensor_copy(ttb[:, k, sb * P : (sb + 1) * P], pt)

            for vb in range(NVB):
                ps = psum_mm.tile([P, 2, TBW], f32)
                # k outer / h inner: consecutive matmuls share lhsT, so the
                # compiler's ldw-opt elides every second LDWEIGHTS.
                for k in range(KC):
                    for h in range(2):
                        nc.tensor.matmul(
                            ps[:, h, :],
                            vT[:, k, vb * P : (vb + 1) * P],
                            ttb[:, k, h * TBW : (h + 1) * TBW],
                            start=(k == 0),
                            stop=(k == KC - 1),
                        )
                # softplus(x) = ln(exp(x) + 1); this toolchain has no Softplus
                # table, but Exp and Ln live in one table set. The bf16 exp
                # scratch also carries the row-max (exp is monotone; host
                # recovers the logit-space max with a log).
                ex = spp.tile([P, 2, TBW], bf16)
                nc.scalar.activation(ex, ps, EXP)
                sp = spp.tile([P, 2, TBW], bf16, tag="sp_dead")
                nc.scalar.activation(
                    sp, ex, LN, bias=1.0, accum_out=rs_cols[:, vb, tb : tb + 1]
                )
                nc.vector.tensor_reduce(
                    mx_cols[:, vb, tb : tb + 1],
                    ex,
                    axis=AXY,
                    op=mybir.AluOpType.max,
                )

        rs_out = singles.tile([P, NVB], f32)
        mx_out = singles.tile([P, NVB], f32)
        for vb in range(NVB):
            nc.vector.reduce_sum(rs_out[:, vb : vb + 1], rs_cols[:, vb, :], axis=AX)
            nc.vector.reduce_max(mx_out[:, vb : vb + 1], mx_cols[:, vb, :], axis=AX)
        nc.sync.dma_start(out=rs_d.ap(), in_=rs_out)
        nc.sync.dma_start(out=rm_d.ap(), in_=mx_out)

    _elide_duplicate_ldweights(nc, mybir)
    nc.compile()
    return nc


def _elide_duplicate_ldweights(nc, mybir):
    """Drop an LDWEIGHTS that reloads the exact weights the PE already holds.

    The Tile lowering emits one LDWEIGHTS per matmul; with the k-outer /
    h-inner loop order every second matmul reuses the same stationary
    operand, so its reload is pure overhead (~110ns on the PE queue each).
    Only sync-free duplicates that immediately follow PE instructions with
    no intervening weight change are removed.
    """

    def _sig(ins):
        ap = ins.ins[-1]
        return repr(ap)

    removed = 0
    for f in nc.m.functions:
        for bb in f.blocks:
            last_sig = None
            keep = []
            for ins in bb.instructions:
                eng = getattr(ins, "engine", None)
                if eng != mybir.EngineType.PE:
                    keep.append(ins)
                    continue
                if isinstance(ins, mybir.InstLdweights):
                    si = ins.sync_info
                    clean = si is None or (
                        len(si.on_wait) == 0 and len(si.on_update) == 0
                    )
                    sig = _sig(ins)
                    if clean and sig == last_sig:
                        removed += 1
                        continue
                    last_sig = sig
                    keep.append(ins)
                elif isinstance(ins, mybir.InstMatmult):
                    keep.append(ins)  # matmul does not disturb loaded weights
                else:
                    last_sig = None
                    keep.append(ins)
            bb.instructions = keep
    return removed


def _get_compiled():
    global _COMPILED
    if _COMPILED is None:
        _COMPILED = _build_nc()
    return _COMPILED


def _run_device(v64, t64, inv_v, inv_t, trace=False):
    from concourse.bass_utils import run_bass_kernel_spmd

    nc = _get_compiled()
    v32 = v64.astype(np.float32)
    t32 = t64.astype(np.float32)
    in_maps = []
    for c in range(NCORES):
        iv = inv_v[c * NV : (c + 1) * NV]  # [1024]
        in_maps.append(
            {
                "v": np.ascontiguousarray(v32[c * NV : (c + 1) * NV]),
                "t": t32,
                # [p, vb] layout: row vb*128+p
                "inv_v": np.ascontiguousarray(
                    iv.reshape(NVB, P).T.astype(np.float32)
                ),
                "inv_t": np.ascontiguousarray(
                    inv_t.reshape(N // P, P).T.astype(np.float32)
                ),
            }
        )
    res = run_bass_kernel_spmd(
        nc, in_maps, core_ids=list(range(NCORES)), trace=trace
    )
    return res


def kernel(video_embed, text_embed, log_logit_scale, _trace=False, _res_out=None):
    video_embed = np.asarray(video_embed)
    text_embed = np.asarray(text_embed)
    scale = float(np.exp(np.float64(np.asarray(log_logit_scale))))

    v64 = video_embed.astype(np.float64)
    t64 = text_embed.astype(np.float64)
    vn = np.linalg.norm(v64, axis=1)
    tn = np.linalg.norm(t64, axis=1)
    inv_v = (scale / vn).astype(np.float64)  # scale folded into v rows
    inv_t = (1.0 / tn).astype(np.float64)

    res = _run_device(v64, t64, inv_v, inv_t, trace=_trace)
    if _res_out is not None:
        _res_out.append(res)

    row_sum = np.concatenate(
        [res.results[c]["row_sum"].T.reshape(-1) for c in range(NCORES)]
    ).astype(np.float64)
    row_max_exp = np.concatenate(
        [res.results[c]["row_max"].T.reshape(-1) for c in range(NCORES)]
    ).astype(np.float64)
    # device tracks max of exp(logit) (monotone); recover logit-space max
    row_max = np.log(np.maximum(row_max_exp, 1e-300))

    # host: exact diag & trace in f64
    v_hat = v64 / vn[:, None]
    t_hat = t64 / tn[:, None]
    diag = scale * np.einsum("ij,ij->i", v_hat, t_hat)
    S = row_sum.sum()
    loss = (S - diag.sum()) / N

    # accuracy: rows where the diagonal could possibly be the argmax get an
    # exact recheck; everything else is confidently not a match.
    cand = np.nonzero(diag >= row_max - MARGIN_BAND)[0]
    k = 0
    for i in cand:
        row = scale * (t_hat @ v_hat[i])
        row[i] = diag[i]
        if int(np.argmax(row)) == i:
            k += 1
    acc = 100.0 * k / N

    return np.float32(loss), np.float32(acc)


# revision 23
# speedup vs baseline: 2.7591x; 1.3250x over previous
"""SigLIP loss kernel for 8 Trainium2 NeuronCores.

Strategy:
  - Row-shard video_embed across the 8 cores (1024 rows each); each core keeps
    the full text_embed in its own HBM (HBM reads beat ring-exchange link BW).
  - Each core computes its 1024x8192 block of logits with bf16 matmuls
    (fp32 PSUM accumulate), applies Softplus on the ScalarEngine with fused
    per-row accumulation, and tracks the per-row max on the VectorEngine.
  - Host: computes inverse row norms (tiny O(N*D) prep), assembles
    loss = (sum softplus(l_ij) - trace) / N, and resolves the argmax-accuracy
    with an exact float64 recheck of the few rows whose diag-vs-rowmax margin
    is within a small band (bf16 can't flip confident rows).
"""

import os
from contextlib import ExitStack

import numpy as np

N, D = 8192, 768
P = 128           # SBUF partitions
KC = D // P       # 6 contraction chunks
NCORES = 8
NV = N // NCORES  # 1024 v rows per core
NVB = NV // P     # 8 v blocks per core
TBW = 512         # t-block width (matmul free dim)
NTB = N // TBW    # 16 t blocks
SUB = TBW // P    # 4 sub-blocks of 128 t rows per t block
MARGIN_BAND = 0.05

_COMPILED = None


def _build_nc():
    import concourse.bass as bass  # noqa: F401
    import concourse.mybir as mybir
    import concourse.tile as tile
    from concourse import bacc

    f32 = mybir.dt.float32
    bf16 = mybir.dt.bfloat16
    EXP = mybir.ActivationFunctionType.Exp
    LN = mybir.ActivationFunctionType.Ln
    AX = mybir.AxisListType.X
    AXY = mybir.AxisListType.XY

    nc = bacc.Bacc(
        "TRN2",
        target_bir_lowering=False,
        debug=False,
        enable_asserts=False,
        num_devices=NCORES,
    )

    # The act-table placement pass picks the first table set containing each
    # function; Exp and Ln individually resolve to different sets, which
    # forces a ~1.3us ACT_TABLE_LOAD per activation. Hide Exp/Ln from every
    # set except the combined one (set order/indices preserved) so both
    # resolve to natural_log_exp_and_others and a single load suffices.
    orig_tables = dict(bacc.get_activation_tables(nc.m.arch))
    patched = {
        name: (fns if name == "natural_log_exp_and_others" else fns - {EXP, LN})
        for name, fns in orig_tables.items()
    }
    bacc.get_activation_tables = lambda arch: patched

    v_d = nc.dram_tensor("v", [NV, D], f32, kind="ExternalInput")
    t_d = nc.dram_tensor("t", [N, D], f32, kind="ExternalInput")
    invv_d = nc.dram_tensor("inv_v", [P, NVB], f32, kind="ExternalInput")
    invt_d = nc.dram_tensor("inv_t", [P, N // P], f32, kind="ExternalInput")
    rs_d = nc.dram_tensor("row_sum", [P, NVB], f32, kind="ExternalOutput")
    rm_d = nc.dram_tensor("row_max", [P, NVB], f32, kind="ExternalOutput")

    with tile.TileContext(nc) as tc, ExitStack() as ctx:
        singles = ctx.enter_context(tc.tile_pool(name="singles", bufs=1))
        vstage = ctx.enter_context(tc.tile_pool(name="vstage", bufs=1))
        tstage = ctx.enter_context(tc.tile_pool(name="tstage", bufs=2))
        tbfp = ctx.enter_context(tc.tile_pool(name="tbfp", bufs=3))
        ttp = ctx.enter_context(tc.tile_pool(name="ttp", bufs=3))
        spp = ctx.enter_context(tc.tile_pool(name="spp", bufs=2))
        psum_mm = ctx.enter_context(tc.tile_pool(name="psum_mm", bufs=3, space="PSUM"))
        psum_tr = ctx.enter_context(tc.tile_pool(name="psum_tr", bufs=2, space="PSUM"))

        from concourse.masks import make_identity

        ident = singles.tile([P, P], bf16)
        make_identity(nc, ident)

        invv = singles.tile([P, NVB], f32)
        nc.gpsimd.dma_start(out=invv, in_=invv_d.ap())
        invt = singles.tile([P, N // P], f32)
        nc.gpsimd.dma_start(out=invt, in_=invt_d.ap())

        # ---- v prep: load, scale+cast to bf16, transpose to [d, row] layout
        # (transposes ride the DMA xbar, keeping the PE free for matmuls)
        vst = vstage.tile([P, NVB, D], f32)
        nc.gpsimd.dma_start(out=vst, in_=v_d.ap().rearrange("(a p) d -> p a d", p=P))
        vbf = vstage.tile([P, NVB, D], bf16)
        for vb in range(NVB):
            nc.vector.tensor_scalar_mul(
                vbf[:, vb, :], vst[:, vb, :], invv[:, vb : vb + 1]
            )
        vT = singles.tile([P, KC, NV], bf16)
        for vb in range(NVB):
            for k in range(KC):
                pt = psum_tr.tile([P, P], bf16)
                nc.tensor.transpose(pt, vbf[:, vb, k * P : (k + 1) * P], ident)
                nc.vector.tensor_copy(vT[:, k, vb * P : (vb + 1) * P], pt)

        NT2 = N // (2 * TBW)  # 8 column-blocks of 1024 t rows
        SUB2 = 2 * TBW // P  # 8 row sub-blocks per column-block
        rs_cols = singles.tile([P, NVB, NT2], f32)
        mx_cols = singles.tile([P, NVB, NT2], bf16)

        # ---- main loop, 2-stage software pipeline: the prep for block tb+1
        # (cast + PE-transpose slices) is emitted BETWEEN the matmul groups of
        # block tb, keeping the PE stream dense (no HAM cool-down) and the
        # DVE/ACT work spread evenly.
        ttb_tiles = {}

        def prep_dma(tb):
            tst = tstage.tile([P, SUB2, D], f32, tag="tst")
            nc.gpsimd.dma_start(
                out=tst,
                in_=t_d.ap()[tb * 1024 : (tb + 1) * 1024, :].rearrange(
                    "(a p) d -> p a d", p=P
                ),
            )
            ttb_tiles[tb] = (
                tst,
                ttp.tile([P, KC, 2 * TBW], bf16, tag="ttb", name=f"ttb{tb}"),
            )

        def prep_slice(tb, sb):
            tst, ttb = ttb_tiles[tb]
            idx = tb * SUB2 + sb
            tb16 = tbfp.tile([P, D], bf16, tag="tb16")
            nc.vector.tensor_scalar_mul(tb16, tst[:, sb, :], invt[:, idx : idx + 1])
            for k in range(KC):
                pt = psum_tr.tile([P, P], bf16)
                nc.tensor.transpose(pt, tb16[:, k * P : (k + 1) * P], ident)
                nc.vector.tensor_copy(ttb[:, k, sb * P : (sb + 1) * P], pt)

        prep_dma(0)
        for sb in range(SUB2):
            prep_slice(0, sb)

        for tb in range(NT2):
            ttb = ttb_tiles.pop(tb)[1]
            if tb + 1 < NT2:
                prep_dma(tb + 1)
            for vb in range(NVB):
                ps = psum_mm.tile([P, 2, TBW], f32)
                # k outer / h inner: consecutive matmuls share lhsT, so the
                # duplicate-LDWEIGHTS elision pass can drop every second load.
                for k in range(KC):
                    for h in range(2):
                        nc.tensor.matmul(
                            ps[:, h, :],
                            vT[:, k, vb * P : (vb + 1) * P],
                            ttb[:, k, h * TBW : (h + 1) * TBW],
                            start=(k == 0),
                            stop=(k == KC - 1),
                        )
                # softplus(x) = ln(exp(x) + 1); this toolchain has no Softplus
                # table, but Exp and Ln live in one table set. The bf16 exp
                # scratch also carries the row-max (exp is monotone; host
                # recovers the logit-space max with a log).
                ex = spp.tile([P, 2, TBW], bf16)
                nc.scalar.activation(ex, ps, EXP)
                sp = spp.tile([P, 2, TBW], bf16, tag="sp_dead")
                nc.scalar.activation(
                    sp, ex, LN, bias=1.0, accum_out=rs_cols[:, vb, tb : tb + 1]
                )
                nc.vector.tensor_reduce(
                    mx_cols[:, vb, tb : tb + 1],
                    ex,
                    axis=AXY,
                    op=mybir.AluOpType.max,
                )
                if tb + 1 < NT2:
                    prep_slice(tb + 1, vb)

        rs_out = singles.tile([P, NVB], f32)
        mx_out = singles.tile([P, NVB], f32)
        for vb in range(NVB):
            nc.vector.reduce_sum(rs_out[:, vb : vb + 1], rs_cols[:, vb, :], axis=AX)
            nc.vector.reduce_max(mx_out[:, vb : vb + 1], mx_cols[:, vb, :], axis=AX)
        nc.sync.dma_start(out=rs_d.ap(), in_=rs_out)
        nc.sync.dma_start(out=rm_d.ap(), in_=mx_out)

    _elide_duplicate_ldweights(nc, mybir)
    nc.compile()
    return nc


def _elide_duplicate_ldweights(nc, mybir):
    """Drop an LDWEIGHTS that reloads the exact weights the PE already holds.

    The Tile lowering emits one LDWEIGHTS per matmul; with the k-outer /
    h-inner loop order every second matmul reuses the same stationary
    operand, so its reload is pure overhead (~110ns on the PE queue each).
    Only sync-free duplicates that immediately follow PE instructions with
    no intervening weight change are removed.
    """

    def _sig(ins):
        ap = ins.ins[-1]
        return repr(ap)

    removed = 0
    for f in nc.m.functions:
        for bb in f.blocks:
            last_sig = None
            keep = []
            for ins in bb.instructions:
                eng = getattr(ins, "engine", None)
                if eng != mybir.EngineType.PE:
                    keep.append(ins)
                    continue
                if isinstance(ins, mybir.InstLdweights):
                    si = ins.sync_info
                    clean = si is None or (
                        len(si.on_wait) == 0 and len(si.on_update) == 0
                    )
                    sig = _sig(ins)
                    if clean and sig == last_sig:
                        removed += 1
                        continue
                    last_sig = sig
                    keep.append(ins)
                elif isinstance(ins, mybir.InstMatmult):
                    keep.append(ins)  # matmul does not disturb loaded weights
                else:
                    last_sig = None
                    keep.append(ins)
            bb.instructions = keep
    return removed


def _get_compiled():
    global _COMPILED
    if _COMPILED is None:
        _COMPILED = _build_nc()
    return _COMPILED


def _run_device(v64, t64, inv_v, inv_t, trace=False):
    from concourse.bass_utils import run_bass_kernel_spmd

    nc = _get_compiled()
    v32 = v64.astype(np.float32)
    t32 = t64.astype(np.float32)
    in_maps = []
    for c in range(NCORES):
        iv = inv_v[c * NV : (c + 1) * NV]  # [1024]
        in_maps.append(
            {
                "v": np.ascontiguousarray(v32[c * NV : (c + 1) * NV]),
                "t": t32,
                # [p, vb] layout: row vb*128+p
                "inv_v": np.ascontiguousarray(
                    iv.reshape(NVB, P).T.astype(np.float32)
                ),
                "inv_t": np.ascontiguousarray(
                    inv_t.reshape(N // P, P).T.astype(np.float32)
                ),
            }
        )
    res = run_bass_kernel_spmd(
        nc, in_maps, core_ids=list(range(NCORES)), trace=trace
    )
    return res


def kernel(video_embed, text_embed, log_logit_scale, _trace=False, _res_out=None):
    video_embed = np.asarray(video_embed)
    text_embed = np.asarray(text_embed)
    scale = float(np.exp(np.float64(np.asarray(log_logit_scale))))

    v64 = video_embed.astype(np.float64)
    t64 = text_embed.astype(np.float64)
    vn = np.linalg.norm(v64, axis=1)
    tn = np.linalg.norm(t64, axis=1)
    inv_v = (scale / vn).astype(np.float64)  # scale folded into v rows
    inv_t = (1.0 / tn).astype(np.float64)

    res = _run_device(v64, t64, inv_v, inv_t, trace=_trace)
    if _res_out is not None:
        _res_out.append(res)

    row_sum = np.concatenate(
        [res.results[c]["row_sum"].T.reshape(-1) for c in range(NCORES)]
    ).astype(np.float64)
    row_max_exp = np.concatenate(
        [res.results[c]["row_max"].T.reshape(-1) for c in range(NCORES)]
    ).astype(np.float64)
    # device tracks max of exp(logit) (monotone); recover logit-space max
    row_max = np.log(np.maximum(row_max_exp, 1e-300))

    # host: exact diag & trace in f64
    v_hat = v64 / vn[:, None]
    t_hat = t64 / tn[:, None]
    diag = scale * np.einsum("ij,ij->i", v_hat, t_hat)
    S = row_sum.sum()
    loss = (S - diag.sum()) / N

    # accuracy: rows where the diagonal could possibly be the argmax get an
    # exact recheck; everything else is confidently not a match.
    cand = np.nonzero(diag >= row_max - MARGIN_BAND)[0]
    k = 0
    for i in cand:
        row = scale * (t_hat @ v_hat[i])
        row[i] = diag[i]
        if int(np.argmax(row)) == i:
            k += 1
    acc = 100.0 * k / N

    return np.float32(loss), np.float32(acc)
